# revision 21
# baseline (speedup 1.0000x reference)
"""Trainium2 Bass kernel for the EvolutionBank scatter+temporal-consistency op.

Math per selected row i (idx unique):
    p = ptr[idx[i]] % 6
    window = bank[idx[i]]            # (6, 32)
    window[p] = emb[i]               # circular-buffer write
    v_w = window / max(||window||, eps)
    sim_q = <v_q, v_{q+1}>,  q = 0..4
    out[i] = 1 / (1 + std(sim, ddof=1))

Distribution: the B=200k referenced rows are sharded across 8 cores. On
the host, each core's 25k rows are routed into 6 buckets by their write
slot p (expert-parallel routing, padded to a fixed 4480 capacity), so
each device tile has a *static* replaced slot: the scatter becomes a
static slot substitution in the access patterns. The overwritten bank
slot is dead data and is dropped during host routing (rows ship as the
5 surviving slots). All shipped data is fp16 (rel tolerance is 2e-2;
fp16 keeps the result within ~1e-3).

v4 engine plan (per tile; R=35 rows/partition). Reductions are 5-level
pairwise fold trees (fp16 TT add measures ~0.58 ns/elem — the DVE 2x
mode — vs tensor_reduce's 1.04 with no fp16 fast path). GPS is left
IDLE on purpose: its Q7 cores stream SBUF so aggressively that
concurrent DVE ops measured 3-25x slower (a 175-elem multiply took
6.5us exactly spanning a GPS fold slice), costing far more DVE time
than GPS contributed:
  ACT  : squares of the 6 merged-window slots -> sqc; tail scalars
         (sqrt(den2), relu(var4), sqrt, +1); odd-tile bank loads
  DVE  : adjacent products -> prc, product fold tree, squares fold
         tree, sim = dot/den (TT divide beats the 1.2us reciprocal),
         std tail, final reciprocal
  SP   : even bank loads, emb loads, stores. ACT ring: odd bank loads.

Software pipeline per step t (stall-free by construction):
  DVE: den2_{t-2} | PR_t | PF_t | SQF_{t-1} | tail1_{t-2} | cons_{t-3}
  ACT: oddload_{t+2} | den_{t-2} | relu/sqrt/u_{t-3} | SQ_t
Raw Bass with manual semaphores; every DVE op incs dve_self and
same-engine RAW dependents wait on it (DVE writes land after the next
op issues otherwise); ACT's relu->sqrt->add chain interlocks on
act_self the same way.
"""

import os
import sys

for _p in ("/opt/trn_rl_repo", os.path.expanduser("~/.axon_site/_ro/trn_rl_repo")):
    if os.path.isdir(_p) and _p not in sys.path:
        sys.path.insert(0, _p)

import numpy as np

NUM_NODES = 1_000_000
W = 6
D = 32
B = 200_000
NCORES = 8
PER = B // NCORES            # 25000 rows per core
RPP = 35                     # rows per partition per tile
CAP = 128 * RPP              # 4480 padded bucket capacity (max bucket 4299)
NT = W                       # one tile per bucket
NP = W - 1                   # 5 product groups

N_RUNS = int(os.environ.get("EVO_RUNS", "2"))  # >=2: first run is warmup
DIV = os.environ.get("EVO_DIV", "0") == "1"   # sim via TT divide (DVE ISA rejects)
DEBUG = os.environ.get("EVO_DEBUG", "0") == "1"  # dump sred/pred per tile

_prog = None
LAST_RESULTS = None


def _build():
    global _prog
    if _prog is not None:
        return _prog

    from contextlib import ExitStack

    import concourse.bass as bass
    from concourse import mybir

    f16 = mybir.dt.float16
    f32 = mybir.dt.float32
    X = mybir.AxisListType.X
    MUL = mybir.AluOpType.mult
    ADD = mybir.AluOpType.add
    SUB = mybir.AluOpType.subtract
    Relu = mybir.ActivationFunctionType.Relu

    nc = bass.Bass(
        detect_race_conditions=os.environ.get("EVO_RACE_DETECT", "0") == "1"
    )
    bank_h = nc.declare_dram_parameter(
        "bank", [NT, 128, RPP, W - 1, D], f16, isOutput=False
    )
    emb_h = nc.declare_dram_parameter(
        "emb", [NT, 128, RPP, 1, D], f16, isOutput=False
    )
    out_h = nc.declare_dram_parameter("out", [NT, 128, RPP], f32, isOutput=True)
    if DEBUG:
        dbg_sred_h = nc.declare_dram_parameter(
            "dbg_sred", [NT, 128, RPP, W], f16, isOutput=True
        )
        dbg_pred_h = nc.declare_dram_parameter(
            "dbg_pred", [NT, 128, RPP, NP], f16, isOutput=True
        )

    TOT = NT

    with ExitStack() as ctx:
        ctx.enter_context(
            nc.allow_low_precision(reason="fp16 pipeline; rel tol is 2e-2")
        )
        block = ctx.enter_context(nc.Block())
        sb = lambda name, shape, dt=f16: ctx.enter_context(
            nc.sbuf_tensor(name, shape, dt)
        )
        sem = lambda name: ctx.enter_context(nc.semaphore(name))

        bank_sb = sb("bank_sb", [128, 2, RPP, W - 1, D])
        emb_sb = sb("emb_sb", [128, 2, RPP, 1, D])
        sqc_sb = sb("sqc_sb", [128, 2, RPP, W, D])     # squares
        sf1_sb = sb("sf1_sb", [128, 2, RPP, W, 16])
        sf2_sb = sb("sf2_sb", [128, 2, RPP, W, 8])
        sf3_sb = sb("sf3_sb", [128, 2, RPP, W, 4])
        sf4_sb = sb("sf4_sb", [128, 2, RPP, W, 2])
        sred_sb = sb("sred_sb", [128, 4, RPP, W])
        prc_sb = sb("prc_sb", [128, 2, RPP, NP, D])    # adjacent products
        pf1_sb = sb("pf1_sb", [128, 2, RPP, NP, 16])
        pf2_sb = sb("pf2_sb", [128, 2, RPP, NP, 8])
        pf3_sb = sb("pf3_sb", [128, 2, RPP, NP, 4])
        pf4_sb = sb("pf4_sb", [128, 2, RPP, NP, 2])
        pred_sb = sb("pred_sb", [128, 4, RPP, NP])
        den2_sb = sb("den2_sb", [128, 2, RPP, NP])
        nd_sb = sb("nd_sb", [128, 2, RPP, NP], f32)
        rec_sb = sb("rec_sb", [128, 2, RPP, NP], f32)
        sim_sb = sb("sim_sb", [128, 2, RPP, NP])
        simsq_sb = sb("simsq_sb", [128, 2, RPP, NP])
        s1_sb = sb("s1_sb", [128, 2, RPP], f32)
        s2_sb = sb("s2_sb", [128, 2, RPP], f32)
        s1sq_sb = sb("s1sq_sb", [128, 2, RPP], f32)
        var4_sb = sb("var4_sb", [128, 2, RPP], f32)
        varc_sb = sb("varc_sb", [128, 2, RPP], f32)
        std_sb = sb("std_sb", [128, 2, RPP], f32)
        u_sb = sb("u_sb", [128, 2, RPP], f32)
        cons_sb = sb("cons_sb", [128, 2, RPP], f32)

        ld_b = [sem(f"ld_b{k}") for k in range(4)]  # bank loads, +16, mod-4
        ld_e = [sem("ld_e0"), sem("ld_e1")]         # emb loads, +16, mod-2
        st2 = [sem("st0"), sem("st1")]              # stores, +16, mod-2
        dbg_st = sem("dbg_st") if DEBUG else None
        act_sq = sem("act_sq")    # +1 per tile: squares done
        act_den = sem("act_den")  # +1 per tile: sqrt(den2) done
        act_u = sem("act_u")      # +1 per tile: relu/sqrt/+1 chain done
        act_self = sem("act_self")  # +1 per interlocked ACT op
        dve_pr = sem("dve_pr")    # +1 per tile: products done (bank/emb free)
        dve_sqf = sem("dve_sqf")  # +1 per tile: sq-fold L1 done (sqc free)
        dve_den2 = sem("dve_den2")  # +1 per tile: den2 written
        dve_t1 = sem("dve_t1")    # +1 per tile: tail1 (var4) done
        dve_c = sem("dve_c")      # +1 per tile: cons done
        dve_self = sem("dve_self")  # +1 per DVE op (same-engine RAW interlock)

        dve_cnt = [0]
        dve_idx = {}

        def dvi(ins, key=None):
            ins.then_inc(dve_self, 1)
            dve_cnt[0] += 1
            if key is not None:
                dve_idx[key] = dve_cnt[0]
            return ins

        def dviw(vector, key):
            tgt = dve_idx.get(key)
            if tgt:
                vector.wait_ge(dve_self, tgt)

        act_cnt = [0]
        act_idx = {}

        def avi(ins, key=None):
            ins.then_inc(act_self, 1)
            act_cnt[0] += 1
            if key is not None:
                act_idx[key] = act_cnt[0]
            return ins

        def aviw(scalar, key):
            tgt = act_idx.get(key)
            if tgt:
                scalar.wait_ge(act_self, tgt)

        # ---------------- SP: even bank loads, emb loads, stores ---------
        @block.sync
        def _(sync):
            for i in range(TOT):
                s = i % 2
                if i >= 2:
                    sync.wait_ge(act_sq, i - 1)
                    sync.wait_ge(dve_pr, i - 1)
                if i % 2 == 0:
                    sync.dma_start(
                        out=bank_sb[:, s], in_=bank_h[i]
                    ).then_inc(ld_b[i % 4], 16)
                sync.dma_start(
                    out=emb_sb[:, s], in_=emb_h[i]
                ).then_inc(ld_e[s], 16)
                if DEBUG and i >= 2:
                    j = i - 2
                    sync.wait_ge(dve_den2, j + 1)  # sred_j/pred_j landed
                    sync.dma_start(
                        out=dbg_sred_h[j], in_=sred_sb[:, j % 4]
                    ).then_inc(dbg_st, 16)
                    sync.dma_start(
                        out=dbg_pred_h[j], in_=pred_sb[:, j % 4]
                    ).then_inc(dbg_st, 16)
                if i >= 3:
                    j = i - 3
                    sync.wait_ge(dve_c, j + 1)
                    sync.dma_start(
                        out=out_h[j], in_=cons_sb[:, j % 2]
                    ).then_inc(st2[j % 2], 16)
            if DEBUG:
                for j in range(max(0, TOT - 2), TOT):
                    sync.wait_ge(dve_den2, j + 1)
                    sync.dma_start(
                        out=dbg_sred_h[j], in_=sred_sb[:, j % 4]
                    ).then_inc(dbg_st, 16)
                    sync.dma_start(
                        out=dbg_pred_h[j], in_=pred_sb[:, j % 4]
                    ).then_inc(dbg_st, 16)
                sync.wait_ge(dbg_st, 16 * 2 * TOT)
            for j in range(max(0, TOT - 3), TOT):
                sync.wait_ge(dve_c, j + 1)
                sync.dma_start(
                    out=out_h[j], in_=cons_sb[:, j % 2]
                ).then_inc(st2[j % 2], 16)
            sync.wait_ge(st2[0], 16 * ((TOT + 1) // 2))
            sync.wait_ge(st2[1], 16 * (TOT // 2))

        # ---------------- ACT: odd loads | den | tail scalars | squares --
        @block.scalar
        def _(scalar):
            for t in range(TOT + 3):
                a, b, c = t, t - 2, t - 3
                io = a + 1
                if io < TOT and io % 2 == 1:
                    # odd bank load on the ACT ring; slot freed by
                    # SQ_{io-2} + PR_{io-2}. The act_sq wait is
                    # load-bearing: dma_start is a SEQ-level op and the
                    # ACT sequencer runs ahead of the ACT engine, so
                    # without it the DMA launches while the engine is
                    # still squaring the slot being overwritten.
                    scalar.wait_ge(act_sq, io - 1)
                    scalar.wait_ge(dve_pr, io - 1)
                    scalar.dma_start(
                        out=bank_sb[:, io % 2], in_=bank_h[io]
                    ).then_inc(ld_b[io % 4], 16)
                if 0 <= b < TOT:
                    scalar.wait_ge(dve_den2, b + 1)
                    if b >= 2:
                        scalar.wait_ge(dve_t1, b - 1)  # nd slot free
                    scalar.sqrt(
                        nd_sb[:, b % 2], den2_sb[:, b % 2]
                    ).then_inc(act_den, 1)
                if 0 <= c < TOT:
                    sc = c % 2
                    scalar.wait_ge(dve_t1, c + 1)
                    if c >= 2:
                        scalar.wait_ge(dve_c, c - 1)  # varc/std/u slots free
                    avi(scalar.activation(
                        varc_sb[:, sc], var4_sb[:, sc], Relu
                    ), key=("relu", c))
                    aviw(scalar, ("relu", c))
                    avi(scalar.sqrt(
                        std_sb[:, sc], varc_sb[:, sc]
                    ), key=("sqstd", c))
                    aviw(scalar, ("sqstd", c))
                    scalar.add(u_sb[:, sc], std_sb[:, sc], 1.0).then_inc(
                        act_u, 1
                    )
                if a < TOT:
                    s = a % 2
                    w = a % NT
                    scalar.wait_ge(ld_b[a % 4], 16 * (a // 4 + 1))
                    scalar.wait_ge(ld_e[s], 16 * (a // 2 + 1))
                    if a >= 2:
                        scalar.wait_ge(dve_sqf, a - 1)  # sqc slot free
                    if w > 0:
                        scalar.square(
                            sqc_sb[:, s, :, 0:w, :], bank_sb[:, s, :, 0:w, :]
                        )
                    if w < W - 1:
                        scalar.square(
                            sqc_sb[:, s, :, w + 1 : W, :],
                            bank_sb[:, s, :, w : W - 1, :],
                        )
                    scalar.square(
                        sqc_sb[:, s, :, w : w + 1, :], emb_sb[:, s]
                    ).then_inc(act_sq, 1)

        # ---------------- DVE: products, folds, tail ---------------------
        @block.vector
        def _(vector):
            for t in range(TOT + 3):
                # a: products + product folds, d: squares fold tree,
                # b: den2/tail1, c: cons
                a, b, c, d = t, t - 2, t - 3, t - 1
                sa = a % 2
                sb_ = b % 2
                sc = c % 2
                sd = d % 2

                # --- den2_b: first op of the step (ACT den_b needs it) ---
                if 0 <= b < TOT:
                    dviw(vector, ("sqf", b))  # sred_b landed (own engine)
                    dviw(vector, ("pf", 4, b))
                    if b >= 2:
                        vector.wait_ge(act_den, b - 1)  # den2 slot free
                    vector.tensor_mul(
                        den2_sb[:, sb_],
                        sred_sb[:, b % 4, :, 0 : W - 1],
                        sred_sb[:, b % 4, :, 1:W],
                    ).then_inc(dve_den2, 1)

                # --- PR_a: adjacent products into prc groups 0..4 --------
                if a < TOT:
                    w = a % NT
                    vector.wait_ge(ld_b[a % 4], 16 * (a // 4 + 1))
                    vector.wait_ge(ld_e[sa], 16 * (a // 2 + 1))
                    prods = []
                    if w >= 2:  # bank-bank pairs q in [0, w-2]
                        prods.append((
                            prc_sb[:, sa, :, 0 : w - 1, :],
                            bank_sb[:, sa, :, 0 : w - 1, :],
                            bank_sb[:, sa, :, 1:w, :],
                        ))
                    if w <= W - 3:  # bank-bank pairs q in [w+1, 4]
                        prods.append((
                            prc_sb[:, sa, :, w + 1 : NP, :],
                            bank_sb[:, sa, :, w : W - 2, :],
                            bank_sb[:, sa, :, w + 1 : W - 1, :],
                        ))
                    if w >= 1:  # pair (w-1, emb)
                        prods.append((
                            prc_sb[:, sa, :, w - 1 : w, :],
                            bank_sb[:, sa, :, w - 1 : w, :],
                            emb_sb[:, sa],
                        ))
                    if w <= W - 2:  # pair (emb, w)
                        prods.append((
                            prc_sb[:, sa, :, w : w + 1, :],
                            emb_sb[:, sa],
                            bank_sb[:, sa, :, w : w + 1, :],
                        ))
                    for k, (out, in0, in1) in enumerate(prods):
                        ins = vector.tensor_mul(out, in0, in1)
                        if k == len(prods) - 1:
                            # single-update cap: the last product signals
                            # dve_pr; PF L1's dve_pr wait covers the RAW
                            ins.then_inc(dve_pr, 1)
                        else:
                            dvi(ins)

                # --- PF_a: fold the 5 product groups (no ACT dep) --------
                if a < TOT:
                    vector.wait_ge(dve_pr, a + 1)  # own products landed
                    levels = [
                        (pf1_sb[:, sa], prc_sb[:, sa, :, :, 0:16],
                         prc_sb[:, sa, :, :, 16:32]),
                        (pf2_sb[:, sa], pf1_sb[:, sa, :, :, 0:8],
                         pf1_sb[:, sa, :, :, 8:16]),
                        (pf3_sb[:, sa], pf2_sb[:, sa, :, :, 0:4],
                         pf2_sb[:, sa, :, :, 4:8]),
                        (pf4_sb[:, sa], pf3_sb[:, sa, :, :, 0:2],
                         pf3_sb[:, sa, :, :, 2:4]),
                        (pred_sb[:, a % 4], pf4_sb[:, sa, :, :, 0],
                         pf4_sb[:, sa, :, :, 1]),
                    ]
                    for li, (out, in0, in1) in enumerate(levels):
                        if li > 0:
                            dviw(vector, ("pf", li - 1, a))
                        dvi(
                            vector.tensor_tensor(
                                out=out, in0=in0, in1=in1, op=ADD
                            ),
                            key=("pf", li, a),
                        )

                # --- SQF_d: squares fold tree (all 6 groups) -------------
                if 0 <= d < TOT:
                    sd = d % 2
                    vector.wait_ge(act_sq, d + 1)
                    # L1 frees the sqc slot: signal ACT via dve_sqf; L2's
                    # standalone dve_sqf wait covers the same-engine RAW
                    vector.tensor_tensor(
                        out=sf1_sb[:, sd],
                        in0=sqc_sb[:, sd, :, :, 0:16],
                        in1=sqc_sb[:, sd, :, :, 16:32],
                        op=ADD,
                    ).then_inc(dve_sqf, 1)
                    vector.wait_ge(dve_sqf, d + 1)
                    dvi(vector.tensor_tensor(
                        out=sf2_sb[:, sd],
                        in0=sf1_sb[:, sd, :, :, 0:8],
                        in1=sf1_sb[:, sd, :, :, 8:16],
                        op=ADD,
                    ), key=("sq2", d))
                    dviw(vector, ("sq2", d))
                    dvi(vector.tensor_tensor(
                        out=sf3_sb[:, sd],
                        in0=sf2_sb[:, sd, :, :, 0:4],
                        in1=sf2_sb[:, sd, :, :, 4:8],
                        op=ADD,
                    ), key=("sq3", d))
                    dviw(vector, ("sq3", d))
                    dvi(vector.tensor_tensor(
                        out=sf4_sb[:, sd],
                        in0=sf3_sb[:, sd, :, :, 0:2],
                        in1=sf3_sb[:, sd, :, :, 2:4],
                        op=ADD,
                    ), key=("sq4", d))
                    dviw(vector, ("sq4", d))
                    dvi(vector.tensor_tensor(
                        out=sred_sb[:, d % 4],
                        in0=sf4_sb[:, sd, :, :, 0],
                        in1=sf4_sb[:, sd, :, :, 1],
                        op=ADD,
                    ), key=("sqf", d))

                # --- tail1_b ---------------------------------------------
                if 0 <= b < TOT:
                    vector.wait_ge(act_den, b + 1)
                    if DIV:
                        dvi(vector.tensor_tensor(
                            out=sim_sb[:, sb_], in0=pred_sb[:, b % 4],
                            in1=nd_sb[:, sb_], op=mybir.AluOpType.divide,
                        ), key=("sim", b))
                    else:
                        dvi(vector.reciprocal(
                            out=rec_sb[:, sb_], in_=nd_sb[:, sb_]
                        ), key=("rec", b))
                        dviw(vector, ("rec", b))
                        dvi(vector.tensor_mul(
                            sim_sb[:, sb_], pred_sb[:, b % 4], rec_sb[:, sb_]
                        ), key=("sim", b))
                    dviw(vector, ("sim", b))
                    dvi(vector.reduce_sum(
                        s1_sb[:, sb_], sim_sb[:, sb_], axis=X
                    ), key=("s1", b))
                    dvi(vector.tensor_mul(
                        simsq_sb[:, sb_], sim_sb[:, sb_], sim_sb[:, sb_]
                    ), key=("simsq", b))
                    dviw(vector, ("simsq", b))
                    dvi(vector.reduce_sum(
                        s2_sb[:, sb_], simsq_sb[:, sb_], axis=X
                    ), key=("s2", b))
                    dviw(vector, ("s1", b))
                    dvi(vector.scalar_tensor_tensor(
                        out=s1sq_sb[:, sb_], in0=s1_sb[:, sb_], scalar=0.05,
                        in1=s1_sb[:, sb_], op0=MUL, op1=MUL,
                    ), key=("s1sq", b))
                    dviw(vector, ("s2", b))
                    dviw(vector, ("s1sq", b))
                    vector.scalar_tensor_tensor(
                        out=var4_sb[:, sb_], in0=s2_sb[:, sb_], scalar=0.25,
                        in1=s1sq_sb[:, sb_], op0=MUL, op1=SUB,
                    ).then_inc(dve_t1, 1)

                # --- cons_c ----------------------------------------------
                if 0 <= c < TOT:
                    vector.wait_ge(act_u, c + 1)
                    if c >= 2:
                        vector.wait_ge(st2[sc], 16 * (c // 2))  # cons free
                    vector.reciprocal(
                        out=cons_sb[:, sc], in_=u_sb[:, sc]
                    ).then_inc(dve_c, 1)

    _prog = nc
    return nc


def _route_inputs(bank, emb, idx_i, ptr_i):
    """Host routing: shard + bucket rows by write slot, pad, pack the 5
    surviving bank slots, cast fp16. Returns (in_maps, metas)."""
    bank2 = np.ascontiguousarray(bank.astype(np.float32, copy=False)).reshape(
        NUM_NODES, W * D
    )
    p_all = (ptr_i[idx_i] % W).astype(np.int64)

    keep_cols = [
        np.array([j for j in range(W) if j != w], dtype=np.int64) for w in range(W)
    ]

    in_maps = []
    metas = []
    for c in range(NCORES):
        sl = slice(c * PER, (c + 1) * PER)
        pc = p_all[sl]
        counts = np.bincount(pc, minlength=W)
        assert counts.max() <= CAP, f"bucket overflow: {counts}"
        order = np.argsort(pc, kind="stable")
        starts = np.zeros(W + 1, np.int64)
        starts[1:] = np.cumsum(counts)
        slot_rows = np.zeros(W * CAP, dtype=np.int64)
        for w in range(W):
            seg = order[starts[w] : starts[w + 1]]
            slot_rows[w * CAP : w * CAP + counts[w]] = seg
            slot_rows[w * CAP + counts[w] : (w + 1) * CAP] = (
                seg[0] if counts[w] > 0 else 0
            )

        g_rows = idx_i[sl][slot_rows]
        rows = bank2[g_rows].reshape(W, CAP, W, D)
        packed = np.empty((W, CAP, W - 1, D), np.float16)
        for w in range(W):
            packed[w] = rows[w][:, keep_cols[w], :]
        emb_c = emb[sl][slot_rows].astype(np.float16)
        in_maps.append(
            {
                "bank": np.ascontiguousarray(packed).reshape(
                    NT, 128, RPP, W - 1, D
                ),
                "emb": np.ascontiguousarray(emb_c).reshape(NT, 128, RPP, 1, D),
            }
        )
        metas.append((slot_rows, counts))
    return in_maps, metas


def kernel(bank, emb, idx, ptr, filled=None, **_unused):
    global LAST_RESULTS
    from concourse.bass_utils import run_bass_kernel_spmd

    nc = _build()

    bank = np.asarray(bank)
    emb = np.asarray(emb, dtype=np.float32)
    idx_i = np.asarray(idx).astype(np.int64)
    ptr_i = np.asarray(ptr).astype(np.int64)
    assert bank.shape == (NUM_NODES, W, D) and emb.shape == (B, D)

    in_maps, metas = _route_inputs(bank, emb, idx_i, ptr_i)

    trace = os.environ.get("EVO_TRACE", "0") == "1"
    res = None
    for _ in range(max(1, N_RUNS)):
        res = run_bass_kernel_spmd(nc, in_maps, list(range(NCORES)), trace=trace)
    LAST_RESULTS = res

    out = np.empty(B, dtype=np.float32)
    for c in range(NCORES):
        cons = np.asarray(res.results[c]["out"]).reshape(W * CAP)
        slot_rows, counts = metas[c]
        for w in range(W):
            n = counts[w]
            out[c * PER + slot_rows[w * CAP : w * CAP + n]] = cons[
                w * CAP : w * CAP + n
            ]
    return out


# revision 27
# speedup vs baseline: 1.2095x; 1.2095x over previous
"""Trainium2 Bass kernel for the EvolutionBank scatter+temporal-consistency op.

Math per selected row i (idx unique):
    p = ptr[idx[i]] % 6
    window = bank[idx[i]]            # (6, 32)
    window[p] = emb[i]               # circular-buffer write
    v_w = window / max(||window||, eps)
    sim_q = <v_q, v_{q+1}>,  q = 0..4
    out[i] = 1 / (1 + std(sim, ddof=1))

Distribution: the B=200k referenced rows are sharded across 8 cores. On
the host, each core's 25k rows are routed into 6 buckets by their write
slot p (expert-parallel routing, padded to a fixed 4480 capacity), so
each device tile has a *static* replaced slot: the scatter becomes a
static slot substitution in the access patterns. The overwritten bank
slot is dropped during host routing; each row ships as the 7-slot
fp16 merged window L = [m_0..m_{w-1}, emb, emb, m_{w+1}..m_5] (emb
duplicated at the write slot), so BOTH the 6 squares and the 5
adjacent products are L-contiguous ranges: squares = L[0:w+1] and
L[w+2:7]; products = L[i]*L[i+1] for i in [0,w) and [w+1,6). One DMA
per tile. fp16 everywhere (rel tolerance is 2e-2; result lands ~6e-5).

v7 engine plan (per tile; R=35 rows/partition). All 11 reductions run
as one merged 5-level pairwise fold tree (fp16 TT add measures ~0.6
ns/elem — the DVE 2x mode — vs tensor_reduce's 1.04 with no fp16 fast
path). GPS is left IDLE on purpose: its Q7 cores stream SBUF so
aggressively that concurrent DVE ops measured 3-25x slower.
  ACT  : squares (2 instrs) -> comb groups 0..5; rsqrt(den2) and
         rsqrt(varc) via a raw Rsqrt InstActivation (bass refuses it
         on accuracy grounds; irrelevant at 2e-2); odd-tile loads
  DVE  : products (2 instrs) -> comb groups 6..10, merged fold tree,
         normalize/std tail, final reciprocal
  SP   : even-tile loads, final merged store.

Software pipeline per step t:
  DVE: den2_{t-2} | PR_t | FL_{t-1} | tail1_{t-2} | tail2_{t-3}
  ACT: oddload_{t+2} | rec_{t-2} | rstd_{t-3} | SQ_t
Raw Bass with manual semaphores; every DVE op incs dve_self and
same-engine RAW dependents wait on it (DVE writes land after the next
op issues otherwise). The act_sq gate on ACT-ring loads is
load-bearing: dma_start is a SEQ-level op and the ACT sequencer runs
ahead of the ACT engine, so an ungated load overwrites the slot the
engine is still squaring.
"""

import os
import sys

for _p in ("/opt/trn_rl_repo", os.path.expanduser("~/.axon_site/_ro/trn_rl_repo")):
    if os.path.isdir(_p) and _p not in sys.path:
        sys.path.insert(0, _p)

import numpy as np

NUM_NODES = 1_000_000
W = 6
D = 32
B = 200_000
NCORES = 8
PER = B // NCORES            # 25000 rows per core
RPP = 35                     # rows per partition per tile
CAP = 128 * RPP              # 4480 padded bucket capacity (max bucket 4299)
NT = W                       # one tile per bucket
NP = W - 1                   # 5 product groups
NG = 2 * W - 1               # 11 reduce groups
LD = W + 1                   # 7 slots per shipped row (emb duplicated)

N_RUNS = int(os.environ.get("EVO_RUNS", "2"))  # >=2: first run is warmup

_prog = None
LAST_RESULTS = None


def _build():
    global _prog
    if _prog is not None:
        return _prog

    from contextlib import ExitStack

    import concourse.bass as bass
    from concourse import mybir

    f16 = mybir.dt.float16
    f32 = mybir.dt.float32
    X = mybir.AxisListType.X
    MUL = mybir.AluOpType.mult
    ADD = mybir.AluOpType.add
    SUB = mybir.AluOpType.subtract

    nc = bass.Bass(
        detect_race_conditions=os.environ.get("EVO_RACE_DETECT", "0") == "1"
    )
    tile_h = nc.declare_dram_parameter(
        "tile", [NT, 128, RPP, LD * D], f16, isOutput=False
    )
    out_h = nc.declare_dram_parameter("out", [NT, 128, RPP], f32, isOutput=True)

    TOT = NT

    with ExitStack() as ctx:
        ctx.enter_context(
            nc.allow_low_precision(reason="fp16 pipeline; rel tol is 2e-2")
        )
        block = ctx.enter_context(nc.Block())
        sb = lambda name, shape, dt=f16: ctx.enter_context(
            nc.sbuf_tensor(name, shape, dt)
        )
        sem = lambda name: ctx.enter_context(nc.semaphore(name))

        tile_sb = sb("tile_sb", [128, 3, RPP, LD * D])
        comb_sb = sb("comb_sb", [128, 2, RPP, NG, D])
        f1_sb = sb("f1_sb", [128, 2, RPP, NG, 16])
        f2_sb = sb("f2_sb", [128, 2, RPP, NG, 8])
        f3_sb = sb("f3_sb", [128, 2, RPP, NG, 4])
        f4_sb = sb("f4_sb", [128, 2, RPP, NG, 2])
        red_sb = sb("red_sb", [128, 4, RPP, NG])
        den2_sb = sb("den2_sb", [128, 2, RPP, NP])
        rec_sb = sb("rec_sb", [128, 2, RPP, NP], f32)
        sim_sb = sb("sim_sb", [128, 2, RPP, NP])
        simsq_sb = sb("simsq_sb", [128, 2, RPP, NP])
        s1_sb = sb("s1_sb", [128, 2, RPP], f32)
        s2_sb = sb("s2_sb", [128, 2, RPP], f32)
        s1sq_sb = sb("s1sq_sb", [128, 2, RPP], f32)
        var4_sb = sb("var4_sb", [128, 2, RPP], f32)
        varc_sb = sb("varc_sb", [128, 2, RPP], f32)
        std_sb = sb("std_sb", [128, 2, RPP], f32)
        rstd_sb = sb("rstd_sb", [128, 2, RPP], f32)
        u_sb = sb("u_sb", [128, 2, RPP], f32)
        cons_sb = sb("cons_sb", [128, NT, RPP], f32)

        ld4 = [sem(f"ld{k}") for k in range(4)]   # tile loads, +16, mod-4
        st_f = sem("st_f")                         # final store, +16
        act_sq = sem("act_sq")    # +1 per tile: squares done
        act_rec = sem("act_rec")  # +1 per tile: rsqrt(den2) done
        act_rs = sem("act_rs")    # +1 per tile: rsqrt(varc) done
        dve_pr = sem("dve_pr")    # +1 per tile: products done
        dve_sqf = sem("dve_sqf")  # +1 per tile: fold L1 done (comb free)
        dve_den2 = sem("dve_den2")  # +1 per tile: den2 written
        dve_t1 = sem("dve_t1")    # +1 per tile: tail1 (varc) done
        dve_c = sem("dve_c")      # +1 per tile: cons done
        dve_self = sem("dve_self")  # +1 per DVE op (same-engine RAW interlock)

        dve_cnt = [0]
        dve_idx = {}

        def dvi(ins, key=None):
            ins.then_inc(dve_self, 1)
            dve_cnt[0] += 1
            if key is not None:
                dve_idx[key] = dve_cnt[0]
            return ins

        def dviw(vector, key):
            tgt = dve_idx.get(key)
            if tgt:
                vector.wait_ge(dve_self, tgt)

        def act_rsqrt(scalar, out, in_):
            # raw InstActivation: bass's activation() refuses Rsqrt on
            # accuracy grounds; our tolerance is 2e-2 so the table's
            # error is irrelevant. Mirrors BassScalarEngine.activation.
            bias = nc.const_aps.scalar_like(0.0, in_)
            return scalar.add_instruction(
                mybir.InstActivation(
                    name=nc.get_next_instruction_name(),
                    func=mybir.ActivationFunctionType.Rsqrt,
                    ins=[
                        scalar.lower_ap(in_),
                        scalar.lower_ap(bias),
                        mybir.ImmediateValue(
                            dtype=mybir.dt.float32, value=1.0
                        ),
                        mybir.ImmediateValue(
                            dtype=mybir.dt.float32, value=0.0
                        ),
                    ],
                    outs=[scalar.lower_ap(out)],
                )
            )

        def lview(s3):
            # [128, RPP, 7, 32] merged-window view of a tile slot
            return tile_sb[:, s3].rearrange("p r (g d) -> p r g d", d=D)

        # ---------------- SP: even-tile loads + final store --------------
        @block.sync
        def _(sync):
            for i in range(0, TOT, 2):
                if i >= 3:
                    # triple-buffered: slot free once tile i-3's readers
                    # are done
                    sync.wait_ge(act_sq, i - 2)
                    sync.wait_ge(dve_pr, i - 2)
                sync.dma_start(
                    out=tile_sb[:, i % 3], in_=tile_h[i]
                ).then_inc(ld4[i % 4], 16)
            sync.wait_ge(dve_c, TOT)
            sync.dma_start(
                out=out_h[:, :, :].rearrange("t p r -> p t r"),
                in_=cons_sb[:, :, :],
            ).then_inc(st_f, 16)
            sync.wait_ge(st_f, 16)

        # ---------------- ACT: odd loads | rec | rstd | squares ----------
        @block.scalar
        def _(scalar):
            for t in range(TOT + 3):
                a, b, c = t, t - 2, t - 3
                io = a + 1
                if io < TOT and io % 2 == 1:
                    # odd tile load on the ACT ring. act_sq gate is
                    # load-bearing (SEQ runs ahead of the engine).
                    if io >= 3:
                        scalar.wait_ge(act_sq, io - 2)
                        scalar.wait_ge(dve_pr, io - 2)
                    scalar.dma_start(
                        out=tile_sb[:, io % 3], in_=tile_h[io]
                    ).then_inc(ld4[io % 4], 16)
                if 0 <= b < TOT:
                    scalar.wait_ge(dve_den2, b + 1)
                    if b >= 2:
                        scalar.wait_ge(dve_t1, b - 1)  # rec slot free
                    act_rsqrt(
                        scalar, rec_sb[:, b % 2], den2_sb[:, b % 2]
                    ).then_inc(act_rec, 1)
                if 0 <= c < TOT:
                    sc = c % 2
                    scalar.wait_ge(dve_t1, c + 1)
                    if c >= 2:
                        scalar.wait_ge(dve_c, c - 1)  # rstd slot free
                    act_rsqrt(
                        scalar, rstd_sb[:, sc], varc_sb[:, sc]
                    ).then_inc(act_rs, 1)
                if a < TOT:
                    s2_ = a % 2
                    w = a % NT
                    L = lview(a % 3)
                    scalar.wait_ge(ld4[a % 4], 16 * (a // 4 + 1))
                    if a >= 2:
                        scalar.wait_ge(dve_sqf, a - 1)  # comb 0..5 free
                    # squares: L[0:w+1] covers slots 0..w (incl emb),
                    # L[w+2:7] covers slots w+1..5 (via the emb dup)
                    last = scalar.square(
                        comb_sb[:, s2_, :, 0 : w + 1, :],
                        L[:, :, 0 : w + 1, :],
                    )
                    if w < W - 1:
                        last = scalar.square(
                            comb_sb[:, s2_, :, w + 1 : W, :],
                            L[:, :, w + 2 : LD, :],
                        )
                    last.then_inc(act_sq, 1)

        # ---------------- DVE: products, merged folds, tail --------------
        @block.vector
        def _(vector):
            for t in range(TOT + 3):
                # a: products, d: merged fold tree, b: den2/tail1, c: tail2
                a, b, c, d = t, t - 2, t - 3, t - 1
                sb_ = b % 2
                sc = c % 2

                # --- den2_b: first op of the step (ACT rec_b needs it) ---
                if 0 <= b < TOT:
                    dviw(vector, ("fl", 4, b))
                    if b >= 2:
                        vector.wait_ge(act_rec, b - 1)  # den2 slot free
                    vector.tensor_mul(
                        den2_sb[:, sb_],
                        red_sb[:, b % 4, :, 0 : W - 1],
                        red_sb[:, b % 4, :, 1:W],
                    ).then_inc(dve_den2, 1)

                # --- PR_a: adjacent products into comb groups 6..10 ------
                if a < TOT:
                    w = a % NT
                    sa2 = a % 2
                    L = lview(a % 3)
                    vector.wait_ge(ld4[a % 4], 16 * (a // 4 + 1))
                    prods = []
                    if w >= 1:  # pairs q in [0, w-1]
                        prods.append((
                            comb_sb[:, sa2, :, W : W + w, :],
                            L[:, :, 0:w, :],
                            L[:, :, 1 : w + 1, :],
                        ))
                    if w <= W - 2:  # pairs q in [w, 4]
                        prods.append((
                            comb_sb[:, sa2, :, W + w : NG, :],
                            L[:, :, w + 1 : W, :],
                            L[:, :, w + 2 : LD, :],
                        ))
                    for k, (out, in0, in1) in enumerate(prods):
                        ins = vector.tensor_mul(out, in0, in1)
                        if k == len(prods) - 1:
                            # single-update cap: last product signals
                            # dve_pr; FL L1's dve_pr wait covers the RAW
                            ins.then_inc(dve_pr, 1)
                        else:
                            dvi(ins)

                # --- FL_d: merged fold tree over all 11 groups -----------
                if 0 <= d < TOT:
                    sd = d % 2
                    vector.wait_ge(act_sq, d + 1)
                    vector.wait_ge(dve_pr, d + 1)
                    # L1 frees the comb slot: signal ACT via dve_sqf;
                    # the standalone dve_sqf wait before L2 covers the
                    # same-engine RAW on f1
                    vector.tensor_tensor(
                        out=f1_sb[:, sd],
                        in0=comb_sb[:, sd, :, :, 0:16],
                        in1=comb_sb[:, sd, :, :, 16:32],
                        op=ADD,
                    ).then_inc(dve_sqf, 1)
                    vector.wait_ge(dve_sqf, d + 1)
                    dvi(vector.tensor_tensor(
                        out=f2_sb[:, sd],
                        in0=f1_sb[:, sd, :, :, 0:8],
                        in1=f1_sb[:, sd, :, :, 8:16],
                        op=ADD,
                    ), key=("fl", 1, d))
                    dviw(vector, ("fl", 1, d))
                    dvi(vector.tensor_tensor(
                        out=f3_sb[:, sd],
                        in0=f2_sb[:, sd, :, :, 0:4],
                        in1=f2_sb[:, sd, :, :, 4:8],
                        op=ADD,
                    ), key=("fl", 2, d))
                    dviw(vector, ("fl", 2, d))
                    dvi(vector.tensor_tensor(
                        out=f4_sb[:, sd],
                        in0=f3_sb[:, sd, :, :, 0:2],
                        in1=f3_sb[:, sd, :, :, 2:4],
                        op=ADD,
                    ), key=("fl", 3, d))
                    dviw(vector, ("fl", 3, d))
                    dvi(vector.tensor_tensor(
                        out=red_sb[:, d % 4],
                        in0=f4_sb[:, sd, :, :, 0],
                        in1=f4_sb[:, sd, :, :, 1],
                        op=ADD,
                    ), key=("fl", 4, d))

                # --- tail1_b ---------------------------------------------
                if 0 <= b < TOT:
                    vector.wait_ge(act_rec, b + 1)
                    dvi(vector.tensor_mul(
                        sim_sb[:, sb_],
                        red_sb[:, b % 4, :, W:NG],
                        rec_sb[:, sb_],
                    ), key=("sim", b))
                    dviw(vector, ("sim", b))
                    dvi(vector.reduce_sum(
                        s1_sb[:, sb_], sim_sb[:, sb_], axis=X
                    ), key=("s1", b))
                    dvi(vector.tensor_mul(
                        simsq_sb[:, sb_], sim_sb[:, sb_], sim_sb[:, sb_]
                    ), key=("simsq", b))
                    dviw(vector, ("simsq", b))
                    dvi(vector.reduce_sum(
                        s2_sb[:, sb_], simsq_sb[:, sb_], axis=X
                    ), key=("s2", b))
                    dviw(vector, ("s1", b))
                    dvi(vector.scalar_tensor_tensor(
                        out=s1sq_sb[:, sb_], in0=s1_sb[:, sb_], scalar=0.05,
                        in1=s1_sb[:, sb_], op0=MUL, op1=MUL,
                    ), key=("s1sq", b))
                    dviw(vector, ("s2", b))
                    dviw(vector, ("s1sq", b))
                    dvi(vector.scalar_tensor_tensor(
                        out=var4_sb[:, sb_], in0=s2_sb[:, sb_], scalar=0.25,
                        in1=s1sq_sb[:, sb_], op0=MUL, op1=SUB,
                    ), key=("var4", b))
                    dviw(vector, ("var4", b))
                    # floor at 1e-12 (not 0): rstd = rsqrt(varc) must stay
                    # finite so std = varc*rstd underflows to ~0 cleanly
                    vector.tensor_scalar_max(
                        varc_sb[:, sb_], var4_sb[:, sb_], 1e-12
                    ).then_inc(dve_t1, 1)

                # --- tail2_c ---------------------------------------------
                if 0 <= c < TOT:
                    vector.wait_ge(act_rs, c + 1)
                    dvi(vector.scalar_tensor_tensor(
                        out=std_sb[:, sc], in0=varc_sb[:, sc], scalar=1.0,
                        in1=rstd_sb[:, sc], op0=MUL, op1=MUL,
                    ), key=("std", c))
                    dviw(vector, ("std", c))
                    dvi(vector.tensor_scalar_add(
                        u_sb[:, sc], std_sb[:, sc], 1.0
                    ), key=("u", c))
                    dviw(vector, ("u", c))
                    vector.reciprocal(
                        out=cons_sb[:, c], in_=u_sb[:, sc]
                    ).then_inc(dve_c, 1)

    _prog = nc
    return nc


def _route_inputs(bank, emb, idx_i, ptr_i):
    """Host routing: shard + bucket rows by write slot, pad, pack each row
    as the 7-slot merged window [m_0..m_{w-1}, emb, emb, m_{w+1}..m_5]
    (emb duplicated), cast fp16. Returns (in_maps, metas)."""
    bank2 = np.ascontiguousarray(bank.astype(np.float32, copy=False)).reshape(
        NUM_NODES, W * D
    )
    p_all = (ptr_i[idx_i] % W).astype(np.int64)

    keep_cols = [
        np.array([j for j in range(W) if j != w], dtype=np.int64) for w in range(W)
    ]

    in_maps = []
    metas = []
    for c in range(NCORES):
        sl = slice(c * PER, (c + 1) * PER)
        pc = p_all[sl]
        counts = np.bincount(pc, minlength=W)
        assert counts.max() <= CAP, f"bucket overflow: {counts}"
        order = np.argsort(pc, kind="stable")
        starts = np.zeros(W + 1, np.int64)
        starts[1:] = np.cumsum(counts)
        slot_rows = np.zeros(W * CAP, dtype=np.int64)
        for w in range(W):
            seg = order[starts[w] : starts[w + 1]]
            slot_rows[w * CAP : w * CAP + counts[w]] = seg
            slot_rows[w * CAP + counts[w] : (w + 1) * CAP] = (
                seg[0] if counts[w] > 0 else 0
            )

        g_rows = idx_i[sl][slot_rows]
        rows = bank2[g_rows].reshape(W, CAP, W, D)
        emb_c = emb[sl][slot_rows].reshape(W, CAP, 1, D).astype(np.float16)
        packed = np.empty((W, CAP, LD, D), np.float16)
        for w in range(W):
            surv = rows[w][:, keep_cols[w], :]  # (CAP, 5, D)
            packed[w, :, 0:w] = surv[:, 0:w]
            packed[w, :, w] = emb_c[w, :, 0]
            packed[w, :, w + 1] = emb_c[w, :, 0]
            packed[w, :, w + 2 :] = surv[:, w:]
        in_maps.append(
            {
                "tile": np.ascontiguousarray(packed).reshape(
                    NT, 128, RPP, LD * D
                ),
            }
        )
        metas.append((slot_rows, counts))
    return in_maps, metas


def kernel(bank, emb, idx, ptr, filled=None, **_unused):
    global LAST_RESULTS
    from concourse.bass_utils import run_bass_kernel_spmd

    nc = _build()

    bank = np.asarray(bank)
    emb = np.asarray(emb, dtype=np.float32)
    idx_i = np.asarray(idx).astype(np.int64)
    ptr_i = np.asarray(ptr).astype(np.int64)
    assert bank.shape == (NUM_NODES, W, D) and emb.shape == (B, D)

    in_maps, metas = _route_inputs(bank, emb, idx_i, ptr_i)

    trace = os.environ.get("EVO_TRACE", "0") == "1"
    res = None
    for _ in range(max(1, N_RUNS)):
        res = run_bass_kernel_spmd(nc, in_maps, list(range(NCORES)), trace=trace)
    LAST_RESULTS = res

    out = np.empty(B, dtype=np.float32)
    for c in range(NCORES):
        cons = np.asarray(res.results[c]["out"]).reshape(W * CAP)
        slot_rows, counts = metas[c]
        for w in range(W):
            n = counts[w]
            out[c * PER + slot_rows[w * CAP : w * CAP + n]] = cons[
                w * CAP : w * CAP + n
            ]
    return out
